# revision 6
# baseline (speedup 1.0000x reference)
"""Trainium2 Bass kernel for nn_MixtureLinear.

Math: out[b,n,o] = sum_{c,r} x[b,n,c] * coef[n,r] * weight[o,c,r]
                   + sum_r coef[n,r] * bias[o,r]

Strategy (8 NeuronCores, token-parallel):
  - Shard tokens N=1024 into 8 slices of NT=128 tokens; each core computes
    out[:, n_lo:n_hi, :] for all batches B=8 -> 1024 output rows per core.
  - Single fat contraction per core: out[row, o] = sum_K z[K, row] * wk[K, o]
    with K = (r, c) of size R*C = 12288, where
      z[(r,c), row=(b,nl)] = x[b, n_lo+nl, c] * coef[n_lo+nl, r]
      wk[(r,c), o]         = weight[o, c, r]
  - z is built on-chip by the vector engine (fp16, 2x mode) as per-r scaled
    copies of the resident x^T slice; the PE accumulates 96 K-chunks of 128
    into fp32 PSUM. bias term (coef @ bias.T) precomputed on host, added by
    DVE when draining PSUM -> SBUF.

kernel(**inputs) takes the FULL numpy inputs and returns the FULL output.
"""

import numpy as np

B, N, C, O, R = 8, 1024, 768, 768, 16
NCORES = 8
NT = N // NCORES          # tokens per core
ROWS = B * NT             # output rows per core (b-major: row = b*NT + nl)
KDIM = R * C              # contraction size
NKC = KDIM // 128         # 96 K-chunks of 128
KC_PER_R = C // 128       # 6 chunks per r
GROUPS = 2                # bn-tiles processed in 2 groups of 4 (PSUM capacity)
GW = ROWS // GROUPS       # 512 rows per group

_BUILT = None             # cached (nc,) so repeated kernel() calls reuse program


def _build_bass():
    import concourse.mybir as mybir
    from concourse import bacc
    from concourse.tile import TileContext

    fp16 = mybir.dt.float16
    fp32 = mybir.dt.float32

    nc = bacc.Bacc("TRN2", target_bir_lowering=False)

    xT_d = nc.dram_tensor("xt", [C, ROWS], fp16, kind="ExternalInput")
    wk_d = nc.dram_tensor("wk", [KDIM, O], fp16, kind="ExternalInput")
    cr_d = nc.dram_tensor("cr", [128, R, GW], fp16, kind="ExternalInput")
    bg_d = nc.dram_tensor("bg", [NT, O], mybir.dt.float32, kind="ExternalInput")
    out_d = nc.dram_tensor("out", [ROWS, O], fp32, kind="ExternalOutput")

    with TileContext(nc) as tc:
        with (
            tc.tile_pool(name="resident", bufs=1) as rpool,
            tc.tile_pool(name="z", bufs=4) as zpool,
            tc.tile_pool(name="osb", bufs=1) as opool,
            tc.tile_pool(name="psum", bufs=1, space="PSUM") as ppool,
        ):
            # x^T resident: [p, cc, row], row-major over (cc*128+p) = c
            xT_s = rpool.tile([128, C // 128, ROWS], fp16, tag="xT")
            nc.sync.dma_start(xT_s, xT_d.ap().rearrange("(t p) n -> p t n", p=128))
            # coef replicated across partitions, tiled over the 4 b's of a group
            cr_s = rpool.tile([128, R, GW], fp16, tag="cr")
            nc.sync.dma_start(cr_s, cr_d.ap())
            # bias_eff rows = n_local -> partition dim
            bg_s = rpool.tile([NT, O], mybir.dt.float32, tag="bg")
            nc.sync.dma_start(bg_s, bg_d.ap())

            # weight, K-major, resident; split into per-r DMA tiles so the PE
            # can start as soon as the first slice lands.
            wkr = wk_d.ap().rearrange("(t p) o -> p t o", p=128)  # [128, 96, O]
            wk_tiles = []
            for i in range(R):
                wt = rpool.tile([128, KC_PER_R, O], fp16, tag=f"wk{i}")
                nc.sync.dma_start(
                    wt, wkr[:, i * KC_PER_R : (i + 1) * KC_PER_R, :]
                )
                wk_tiles.append(wt)

            for g in range(GROUPS):
                psums = [
                    ppool.tile([128, O], fp32, tag=f"ps{t}", name=f"ps_{g}_{t}")
                    for t in range(4)
                ]
                for kc in range(NKC):
                    r, cc = kc // KC_PER_R, kc % KC_PER_R
                    zt = zpool.tile([128, GW], fp16, tag="z")
                    nc.vector.tensor_mul(
                        zt,
                        xT_s[:, cc, g * GW : (g + 1) * GW],
                        cr_s[:, r, :],
                    )
                    wt = wk_tiles[r][:, cc, :]
                    first, last = kc == 0, kc == NKC - 1
                    for t4 in range(4):
                        lhsT = zt[:, t4 * 128 : (t4 + 1) * 128]
                        nc.tensor.matmul(
                            psums[t4][:, 0:512], lhsT, wt[:, 0:512],
                            start=first, stop=last,
                        )
                        nc.tensor.matmul(
                            psums[t4][:, 512:O], lhsT, wt[:, 512:O],
                            start=first, stop=last,
                        )
                for t4 in range(4):
                    # dedicated buffer per output tile: avoids a WAR wait on
                    # the outbound DMA (DVE instructions only have 2 wait slots)
                    osb = opool.tile(
                        [128, O], fp32, tag=f"osb{g}{t4}", name=f"osb_{g}_{t4}"
                    )
                    nc.vector.tensor_add(osb, psums[t4], bg_s)
                    row0 = (g * 4 + t4) * 128
                    nc.sync.dma_start(out_d[row0 : row0 + 128, :], osb)

    nc.compile()
    return nc


def _prep_inputs(x, coef, weight, bias):
    """Host-side shard + repack. Returns per-core input maps."""
    wk = np.ascontiguousarray(
        weight.transpose(2, 1, 0).reshape(KDIM, O)
    ).astype(np.float16)
    bias_eff = (coef @ bias.T).astype(np.float32)  # [N, O]

    in_maps = []
    for cid in range(NCORES):
        n_lo = cid * NT
        xs = x[:, n_lo : n_lo + NT, :]  # (B, NT, C)
        xT = np.ascontiguousarray(
            xs.transpose(2, 0, 1).reshape(C, ROWS)
        ).astype(np.float16)
        cf = coef[n_lo : n_lo + NT].astype(np.float16)  # (NT, R)
        inner = np.tile(cf.T, (1, GW // NT))  # [R, GW]
        cr = np.ascontiguousarray(
            np.broadcast_to(inner[None, :, :], (128, R, GW))
        )
        bg = np.ascontiguousarray(bias_eff[n_lo : n_lo + NT])  # (NT, O) fp32
        in_maps.append({"xt": xT, "wk": wk, "cr": cr, "bg": bg})
    return in_maps


def _assemble(results):
    out = np.empty((B, N, O), dtype=np.float32)
    for cid in range(NCORES):
        n_lo = cid * NT
        out[:, n_lo : n_lo + NT, :] = results[cid]["out"].reshape(B, NT, O)
    return out


def _run(x, coef, weight, bias, trace=False, **spmd_kwargs):
    global _BUILT
    from concourse.bass_utils import run_bass_kernel_spmd

    if _BUILT is None:
        _BUILT = _build_bass()
    nc = _BUILT
    in_maps = _prep_inputs(x, coef, weight, bias)
    res = run_bass_kernel_spmd(
        nc, in_maps, core_ids=list(range(NCORES)), trace=trace, **spmd_kwargs
    )
    return _assemble(res.results), res


def kernel(x, coef, weight, bias):
    out, _ = _run(
        np.asarray(x, dtype=np.float32),
        np.asarray(coef, dtype=np.float32),
        np.asarray(weight, dtype=np.float32),
        np.asarray(bias, dtype=np.float32),
    )
    return out


# revision 15
# speedup vs baseline: 115.0849x; 115.0849x over previous
"""Trainium2 Bass kernel for nn_MixtureLinear.

Math: out[b,n,o] = sum_{c,r} x[b,n,c] * coef[n,r] * weight[o,c,r]
                   + sum_r coef[n,r] * bias[o,r]

Strategy (8 NeuronCores, token-parallel):
  - Shard tokens N=1024 into 8 slices of NT=128 tokens; each core computes
    out[:, n_lo:n_hi, :] for all batches B=8 -> 1024 output rows per core.
  - Single fat contraction per core: out[row, o] = sum_K z[K, row] * wk[K, o]
    with K = (r, c) of size R*C = 12288, where
      z[(r,c), row=(b,nl)] = x[b, n_lo+nl, c] * coef[n_lo+nl, r]
      wk[(r,c), o]         = weight[o, c, r]
  - z is built on-chip by the vector engine (fp16, 2x mode) as per-r scaled
    copies of the resident x^T slice; the PE accumulates 96 K-chunks of 128
    into fp32 PSUM. bias term (coef @ bias.T) precomputed on host, added by
    DVE when draining PSUM -> SBUF.

kernel(**inputs) takes the FULL numpy inputs and returns the FULL output.
"""

import numpy as np

B, N, C, O, R = 8, 1024, 768, 768, 16
NCORES = 8
NT = N // NCORES          # tokens per core
ROWS = B * NT             # output rows per core (b-major: row = b*NT + nl)
KDIM = R * C              # contraction size
NKC = KDIM // 128         # 96 K-chunks of 128
KC_PER_R = C // 128       # 6 chunks per r
GROUPS = 2                # bn-tiles processed in 2 groups of 4 (PSUM capacity)
GW = ROWS // GROUPS       # 512 rows per group

_BUILT = None             # cached (nc,) so repeated kernel() calls reuse program


def _build_bass(reps=None):
    import contextlib

    import concourse.mybir as mybir
    from concourse import bacc
    from concourse.tile import TileContext

    fp16 = mybir.dt.float16
    fp32 = mybir.dt.float32

    nc = bacc.Bacc("TRN2", target_bir_lowering=False)

    xT_d = nc.dram_tensor("xt", [C, ROWS], fp16, kind="ExternalInput")
    wk_d = nc.dram_tensor("wk", [KDIM, O], fp16, kind="ExternalInput")
    cr_d = nc.dram_tensor("cr", [128, R * GW], fp16, kind="ExternalInput")
    bg_d = nc.dram_tensor("bg", [NT, O], mybir.dt.float32, kind="ExternalInput")
    out_d = nc.dram_tensor("out", [ROWS, O], fp32, kind="ExternalOutput")

    with TileContext(nc) as tc:
        with (
            tc.tile_pool(name="resident", bufs=1) as rpool,
            tc.tile_pool(name="z", bufs=4) as zpool,
            tc.tile_pool(name="osb", bufs=1) as opool,
            tc.tile_pool(name="psum", bufs=1, space="PSUM") as ppool,
            tc.For_i(0, reps, 1) if reps else contextlib.nullcontext(),
        ):
            # DMA issue order = first-use order (HWDGE ring is FIFO): the PE's
            # kc-th matmul group needs cr[r], xT[cc] (group-0 half) and
            # wk[r][cc]; keep each piece small and just-in-time.
            cr_s = rpool.tile([128, R, GW], fp16, tag="cr")
            crf = cr_d.ap().rearrange("p (r g) -> p r g", g=GW)
            xT_s = rpool.tile([128, C // 128, ROWS], fp16, tag="xT")
            xTr = xT_d.ap().rearrange("(t p) n -> p t n", p=128)
            wkr = wk_d.ap().rearrange("(t p) o -> p t o", p=128)  # [128, 96, O]
            wk_tiles = [
                rpool.tile([128, KC_PER_R, O], fp16, tag=f"wk{i}", name=f"wk_{i}")
                for i in range(R)
            ]

            nc.sync.dma_start(cr_s[:, 0:1, :], crf[:, 0:1, :])
            # group-0 halves of x^T interleaved with the r=0 weight chunks
            for ci in range(C // 128):
                nc.sync.dma_start(
                    xT_s[:, ci : ci + 1, 0:GW], xTr[:, ci : ci + 1, 0:GW]
                )
                nc.sync.dma_start(
                    wk_tiles[0][:, ci : ci + 1, :], wkr[:, ci : ci + 1, :]
                )
            # per-r: coef slice + weight tile, in consumption order
            for i in range(1, R):
                nc.sync.dma_start(cr_s[:, i : i + 1, :], crf[:, i : i + 1, :])
                nc.sync.dma_start(
                    wk_tiles[i], wkr[:, i * KC_PER_R : (i + 1) * KC_PER_R, :]
                )
            # group-1 halves of x^T (needed only after ~kc=96)
            for ci in range(C // 128):
                nc.sync.dma_start(
                    xT_s[:, ci : ci + 1, GW:ROWS], xTr[:, ci : ci + 1, GW:ROWS]
                )
            # bias_eff rows = n_local -> partition dim (needed only at drain)
            bg_s = rpool.tile([NT, O], mybir.dt.float32, tag="bg")
            nc.sync.dma_start(bg_s, bg_d.ap())

            for g in range(GROUPS):
                psums = [
                    ppool.tile([128, O], fp32, tag=f"ps{t}", name=f"ps_{g}_{t}")
                    for t in range(4)
                ]
                for kc in range(NKC):
                    r, cc = kc // KC_PER_R, kc % KC_PER_R
                    zt = zpool.tile([128, GW], fp16, tag="z")
                    nc.vector.tensor_mul(
                        zt,
                        xT_s[:, cc, g * GW : (g + 1) * GW],
                        cr_s[:, r, :],
                    )
                    wt = wk_tiles[r][:, cc, :]
                    first, last = kc == 0, kc == NKC - 1
                    for t4 in range(4):
                        lhsT = zt[:, t4 * 128 : (t4 + 1) * 128]
                        nc.tensor.matmul(
                            psums[t4][:, 0:512], lhsT, wt[:, 0:512],
                            start=first, stop=last,
                        )
                        nc.tensor.matmul(
                            psums[t4][:, 512:O], lhsT, wt[:, 512:O],
                            start=first, stop=last,
                        )
                for t4 in range(4):
                    # dedicated buffer per output tile: avoids a WAR wait on
                    # the outbound DMA (DVE instructions only have 2 wait slots)
                    osb = opool.tile(
                        [128, O], fp32, tag=f"osb{g}{t4}", name=f"osb_{g}_{t4}"
                    )
                    nc.vector.tensor_add(osb, psums[t4], bg_s)
                    row0 = (g * 4 + t4) * 128
                    nc.sync.dma_start(out_d[row0 : row0 + 128, :], osb)

    nc.compile()
    return nc


def _prep_inputs(x, coef, weight, bias):
    """Host-side shard + repack. Returns per-core input maps."""
    wk = np.ascontiguousarray(
        weight.transpose(2, 1, 0).reshape(KDIM, O)
    ).astype(np.float16)
    bias_eff = (coef @ bias.T).astype(np.float32)  # [N, O]

    in_maps = []
    for cid in range(NCORES):
        n_lo = cid * NT
        xs = x[:, n_lo : n_lo + NT, :]  # (B, NT, C)
        xT = np.ascontiguousarray(
            xs.transpose(2, 0, 1).reshape(C, ROWS)
        ).astype(np.float16)
        cf = coef[n_lo : n_lo + NT].astype(np.float16)  # (NT, R)
        inner = np.tile(cf.T, (1, GW // NT))  # [R, GW]
        cr = np.ascontiguousarray(
            np.broadcast_to(inner[None, :, :], (128, R, GW))
        )
        bg = np.ascontiguousarray(bias_eff[n_lo : n_lo + NT])  # (NT, O) fp32
        in_maps.append({"xt": xT, "wk": wk, "cr": cr, "bg": bg})
    return in_maps


def _assemble(results):
    out = np.empty((B, N, O), dtype=np.float32)
    for cid in range(NCORES):
        n_lo = cid * NT
        out[:, n_lo : n_lo + NT, :] = results[cid]["out"].reshape(B, NT, O)
    return out


def _run(x, coef, weight, bias, trace=False, **spmd_kwargs):
    global _BUILT
    from concourse.bass_utils import run_bass_kernel_spmd

    if _BUILT is None:
        _BUILT = _build_bass()
    nc = _BUILT
    in_maps = _prep_inputs(x, coef, weight, bias)
    res = run_bass_kernel_spmd(
        nc, in_maps, core_ids=list(range(NCORES)), trace=trace, **spmd_kwargs
    )
    return _assemble(res.results), res


def kernel(x, coef, weight, bias):
    out, _ = _run(
        np.asarray(x, dtype=np.float32),
        np.asarray(coef, dtype=np.float32),
        np.asarray(weight, dtype=np.float32),
        np.asarray(bias, dtype=np.float32),
    )
    return out


# revision 30
# speedup vs baseline: 115.7761x; 1.0060x over previous
"""Trainium2 Bass kernel for nn_MixtureLinear.

Math: out[b,n,o] = sum_{c,r} x[b,n,c] * coef[n,r] * weight[o,c,r]
                   + sum_r coef[n,r] * bias[o,r]

Strategy (8 NeuronCores, token-parallel):
  - Shard tokens N=1024 into 8 slices of NT=128 tokens; each core computes
    out[:, n_lo:n_hi, :] for all batches B=8 -> 1024 output rows per core.
  - Single fat contraction per core: out[row, o] = sum_K z[K, row] * wk[K, o]
    with K = (r, c) of size R*C = 12288, where
      z[(r,c), row=(b,nl)] = x[b, n_lo+nl, c] * coef[n_lo+nl, r]
      wk[(r,c), o]         = weight[o, c, r]
  - z is built on-chip by the vector engine (fp16, 2x mode) as per-r scaled
    copies of the resident x^T slice; the PE accumulates 96 K-chunks of 128
    into fp32 PSUM. bias term (coef @ bias.T) precomputed on host, added by
    DVE when draining PSUM -> SBUF.

kernel(**inputs) takes the FULL numpy inputs and returns the FULL output.
"""

import numpy as np

B, N, C, O, R = 8, 1024, 768, 768, 16
NCORES = 8
NT = N // NCORES          # tokens per core
ROWS = B * NT             # output rows per core (b-major: row = b*NT + nl)
KDIM = R * C              # contraction size
NKC = KDIM // 128         # 96 K-chunks of 128
KC_PER_R = C // 128       # 6 chunks per r
GROUPS = 2                # bn-tiles processed in 2 groups of 4 (PSUM capacity)
GW = ROWS // GROUPS       # 512 rows per group

_BUILT = None             # cached (nc,) so repeated kernel() calls reuse program


def _build_bass(reps=None, _timing_shared_lhst=False, _timing_no_zbuild=False):
    import contextlib

    import concourse.mybir as mybir
    from concourse import bacc
    from concourse.tile import TileContext

    fp16 = mybir.dt.float16
    fp32 = mybir.dt.float32

    nc = bacc.Bacc("TRN2", target_bir_lowering=False)

    xT_d = nc.dram_tensor("xt", [C, ROWS], fp16, kind="ExternalInput")
    wk_d = nc.dram_tensor("wk", [KDIM, O], fp16, kind="ExternalInput")
    cr_d = nc.dram_tensor("cr", [128, R * GW], fp16, kind="ExternalInput")
    bg_d = nc.dram_tensor("bg", [NT, O], mybir.dt.float32, kind="ExternalInput")
    out_d = nc.dram_tensor("out", [ROWS, O], fp32, kind="ExternalOutput")

    with TileContext(nc) as tc:
        with (
            tc.tile_pool(name="resident", bufs=1) as rpool,
            tc.tile_pool(name="z", bufs=4) as zpool,
            tc.tile_pool(name="osb", bufs=1) as opool,
            tc.tile_pool(name="psum", bufs=1, space="PSUM") as ppool,
            tc.For_i(0, reps, 1) if reps else contextlib.nullcontext(),
        ):
            # DMA issue order = first-use order (HWDGE ring is FIFO): the PE's
            # kc-th matmul group needs cr[r], xT[cc] (group-0 half) and
            # wk[r][cc]; keep each piece small and just-in-time.
            cr_s = rpool.tile([128, R, GW], fp16, tag="cr")
            crf = cr_d.ap().rearrange("p (r g) -> p r g", g=GW)
            xT_s = rpool.tile([128, C // 128, ROWS], fp16, tag="xT")
            xTr = xT_d.ap().rearrange("(t p) n -> p t n", p=128)
            wkr = wk_d.ap().rearrange("(t p) o -> p t o", p=128)  # [128, 96, O]
            wk_tiles = [
                rpool.tile([128, KC_PER_R, O], fp16, tag=f"wk{i}", name=f"wk_{i}")
                for i in range(R)
            ]

            nc.sync.dma_start(cr_s[:, 0:1, :], crf[:, 0:1, :])
            # group-0 halves of x^T interleaved with the r=0 weight chunks
            for ci in range(C // 128):
                nc.sync.dma_start(
                    xT_s[:, ci : ci + 1, 0:GW], xTr[:, ci : ci + 1, 0:GW]
                )
                nc.sync.dma_start(
                    wk_tiles[0][:, ci : ci + 1, :], wkr[:, ci : ci + 1, :]
                )
            # per-r: coef slice + weight tile, in consumption order. Keep the
            # instruction handles: wk[r>=3] is paced against PE progress below
            # to avoid an HBM burst (2 cores share one HBM stack).
            wk_dmas = {}
            for i in range(1, R):
                nc.sync.dma_start(cr_s[:, i : i + 1, :], crf[:, i : i + 1, :])
                wk_dmas[i] = nc.sync.dma_start(
                    wk_tiles[i], wkr[:, i * KC_PER_R : (i + 1) * KC_PER_R, :]
                )
            # group-1 halves of x^T (needed only after ~kc=96)
            for ci in range(C // 128):
                nc.sync.dma_start(
                    xT_s[:, ci : ci + 1, GW:ROWS], xTr[:, ci : ci + 1, GW:ROWS]
                )
            # bias_eff rows = n_local -> partition dim (needed only at drain)
            bg_s = rpool.tile([NT, O], mybir.dt.float32, tag="bg")
            nc.sync.dma_start(bg_s, bg_d.ap())

            first_mm_of_r = {}
            for g in range(GROUPS):
                psums = [
                    ppool.tile([128, O], fp32, tag=f"ps{t}", name=f"ps_{g}_{t}")
                    for t in range(4)
                ]
                for kc in range(NKC):
                    r, cc = kc // KC_PER_R, kc % KC_PER_R
                    if _timing_no_zbuild:
                        zt = xT_s[:, cc, 0:GW]  # wrong data, timing only
                    else:
                        zt = zpool.tile([128, GW], fp16, tag="z")
                        nc.vector.tensor_mul(
                            zt,
                            xT_s[:, cc, g * GW : (g + 1) * GW],
                            cr_s[:, r, :],
                        )
                    wt = wk_tiles[r][:, cc, :]
                    first, last = kc == 0, kc == NKC - 1
                    for t4 in range(4):
                        lt4 = 0 if _timing_shared_lhst else t4
                        lhsT = zt[:, lt4 * 128 : (lt4 + 1) * 128]
                        mm = nc.tensor.matmul(
                            psums[t4][:, 0:512], lhsT, wt[:, 0:512],
                            start=first, stop=last,
                        )
                        if g == 0 and t4 == 0 and cc == 0:
                            first_mm_of_r[r] = mm
                        nc.tensor.matmul(
                            psums[t4][:, 512:O], lhsT, wt[:, 512:O],
                            start=first, stop=last,
                        )
                for t4 in range(4):
                    # dedicated buffer per output tile: avoids a WAR wait on
                    # the outbound DMA (DVE instructions only have 2 wait slots)
                    osb = opool.tile(
                        [128, O], fp32, tag=f"osb{g}{t4}", name=f"osb_{g}_{t4}"
                    )
                    nc.vector.tensor_add(osb, psums[t4], bg_s)
                    row0 = (g * 4 + t4) * 128
                    nc.sync.dma_start(out_d[row0 : row0 + 128, :], osb)

            # Pace the weight stream: wk[r] may only start once the PE has
            # begun consuming r-3 (stays ~3.6 MB ahead instead of bursting
            # all 18.9 MB against the paired core on the shared HBM stack).
            from concourse.tile import add_dep_helper

            LOOKAHEAD = 3
            for i in range(1 + LOOKAHEAD, R):
                add_dep_helper(
                    wk_dmas[i].ins,
                    first_mm_of_r[i - LOOKAHEAD].ins,
                    sync=True,
                    reason="pace wk stream vs PE progress",
                )

    nc.compile()
    return nc


def _build_bass_v2(reps=None):
    """LDW-amortized variant: stationary = weight chunk (576 LDWEIGHTS,
    1024 moving columns each), output transposed [O, ROWS] (host undoes).
    K is split in 2 halves (h) x o in 2 halves (q); each (h,q) pass keeps
    6 one-bank PSUM tiles [o-128, row-512]; h=0 drains to SBUF partials
    (+bias), h=1 adds partials and stores.
    """
    import contextlib

    import concourse.mybir as mybir
    from concourse import bacc
    from concourse.tile import TileContext

    fp16 = mybir.dt.float16
    fp32 = mybir.dt.float32

    nc = bacc.Bacc("TRN2", target_bir_lowering=False)

    xT_d = nc.dram_tensor("xt", [C, ROWS], fp16, kind="ExternalInput")
    wk_d = nc.dram_tensor("wk", [KDIM, O], fp16, kind="ExternalInput")
    cr_d = nc.dram_tensor("cr", [128, R * ROWS], fp16, kind="ExternalInput")
    bt_d = nc.dram_tensor("bt", [O, ROWS], fp16, kind="ExternalInput")
    out_d = nc.dram_tensor("out", [O, ROWS], fp32, kind="ExternalOutput")

    NOT = O // 128          # 6 o-tiles
    HK = NKC // 2           # 48 kc per K-half
    with TileContext(nc) as tc:
        with (
            tc.tile_pool(name="resident", bufs=1) as rpool,
            tc.tile_pool(name="z", bufs=6) as zpool,
            tc.tile_pool(name="wq", bufs=6) as wpool,
            tc.tile_pool(name="pq", bufs=1) as qpool,
            tc.tile_pool(name="osb", bufs=1) as opool,
            tc.tile_pool(name="psum", bufs=1, space="PSUM") as ppool,
            tc.For_i(0, reps, 1) if reps else contextlib.nullcontext(),
        ):
            crf = cr_d.ap().rearrange("p (r n) -> p r n", n=ROWS)
            cr_s = rpool.tile([128, R, ROWS], fp16, tag="cr")
            nc.sync.dma_start(cr_s[:, 0:1, :], crf[:, 0:1, :])
            xT_s = rpool.tile([128, C // 128, ROWS], fp16, tag="xT")
            xTr = xT_d.ap().rearrange("(t p) n -> p t n", p=128)
            for ci in range(C // 128):
                nc.sync.dma_start(xT_s[:, ci : ci + 1, :], xTr[:, ci : ci + 1, :])
            for i in range(1, R):
                nc.sync.dma_start(cr_s[:, i : i + 1, :], crf[:, i : i + 1, :])
            bt_s = rpool.tile([128, NOT, ROWS], fp16, tag="bt")
            nc.sync.dma_start(bt_s, bt_d.ap().rearrange("(t p) n -> p t n", p=128))

            wkr = wk_d.ap().rearrange("(t p) o -> p t o", p=128)  # [128, 96, O]
            partials = {}
            for h in range(2):
                for q in range(2):
                    ps = {
                        (ot, rh): ppool.tile(
                            [128, 512], fp32, tag=f"ps{ot}{rh}",
                            name=f"ps_{h}_{q}_{ot}_{rh}",
                        )
                        for ot in range(3)
                        for rh in range(2)
                    }
                    for j in range(HK):
                        kc = h * HK + j
                        r, cc = kc // KC_PER_R, kc % KC_PER_R
                        zt = zpool.tile([128, ROWS], fp16, tag="z")
                        nc.vector.tensor_mul(zt, xT_s[:, cc, :], cr_s[:, r, :])
                        wq = wpool.tile([128, 1, 384], fp16, tag="wq")
                        nc.sync.dma_start(
                            wq, wkr[:, kc : kc + 1, q * 384 : (q + 1) * 384]
                        )
                        first, last = j == 0, j == HK - 1
                        for ot in range(3):
                            lhsT = wq[:, 0, ot * 128 : (ot + 1) * 128]
                            for rh in range(2):
                                nc.tensor.matmul(
                                    ps[(ot, rh)], lhsT,
                                    zt[:, rh * 512 : (rh + 1) * 512],
                                    start=first, stop=last,
                                )
                    for ot in range(3):
                        for rh in range(2):
                            bslice = bt_s[
                                :, q * 3 + ot, rh * 512 : (rh + 1) * 512
                            ]
                            if h == 0:
                                pq = qpool.tile(
                                    [128, 512], fp32, tag=f"pq{q}{ot}{rh}",
                                    name=f"pq_{q}_{ot}_{rh}",
                                )
                                nc.vector.tensor_add(pq, ps[(ot, rh)], bslice)
                                partials[(q, ot, rh)] = pq
                            else:
                                osb = opool.tile(
                                    [128, 512], fp32, tag=f"osb{q}{ot}{rh}",
                                    name=f"osb_{q}_{ot}_{rh}",
                                )
                                nc.vector.tensor_add(
                                    osb, ps[(ot, rh)], partials[(q, ot, rh)]
                                )
                                o0 = q * 384 + ot * 128
                                nc.sync.dma_start(
                                    out_d[o0 : o0 + 128,
                                          rh * 512 : (rh + 1) * 512],
                                    osb,
                                )

    nc.compile()
    return nc


def _prep_inputs_v2(x, coef, weight, bias):
    wk = np.ascontiguousarray(
        weight.transpose(2, 1, 0).reshape(KDIM, O)
    ).astype(np.float16)
    bias_eff = (coef @ bias.T).astype(np.float32)  # [N, O]

    in_maps = []
    for cid in range(NCORES):
        n_lo = cid * NT
        xs = x[:, n_lo : n_lo + NT, :]
        xT = np.ascontiguousarray(
            xs.transpose(2, 0, 1).reshape(C, ROWS)
        ).astype(np.float16)
        cf = coef[n_lo : n_lo + NT].astype(np.float16)  # (NT, R)
        inner = np.tile(cf.T, (1, ROWS // NT))  # [R, ROWS]
        cr = np.ascontiguousarray(
            np.broadcast_to(inner[None, :, :], (128, R, ROWS))
        ).reshape(128, R * ROWS)
        # bias transposed [O, ROWS], rows b-major repeat
        bt = np.ascontiguousarray(
            np.tile(bias_eff[n_lo : n_lo + NT].T, (1, B))
        ).astype(np.float16)
        # note: rows are (b, nl) b-major -> bias pattern repeats per 128: tile
        # along axis1 B times gives [O, B*NT] with [:, b*NT+nl] = bias[nl, :].T
        in_maps.append({"xt": xT, "wk": wk, "cr": cr, "bt": bt})
    return in_maps


def _assemble_v2(results):
    out = np.empty((B, N, O), dtype=np.float32)
    for cid in range(NCORES):
        n_lo = cid * NT
        out[:, n_lo : n_lo + NT, :] = (
            results[cid]["out"].T.reshape(B, NT, O)
        )
    return out


def _prep_inputs(x, coef, weight, bias):
    """Host-side shard + repack. Returns per-core input maps."""
    wk = np.ascontiguousarray(
        weight.transpose(2, 1, 0).reshape(KDIM, O)
    ).astype(np.float16)
    bias_eff = (coef @ bias.T).astype(np.float32)  # [N, O]

    in_maps = []
    for cid in range(NCORES):
        n_lo = cid * NT
        xs = x[:, n_lo : n_lo + NT, :]  # (B, NT, C)
        xT = np.ascontiguousarray(
            xs.transpose(2, 0, 1).reshape(C, ROWS)
        ).astype(np.float16)
        cf = coef[n_lo : n_lo + NT].astype(np.float16)  # (NT, R)
        inner = np.tile(cf.T, (1, GW // NT))  # [R, GW]
        cr = np.ascontiguousarray(
            np.broadcast_to(inner[None, :, :], (128, R, GW))
        )
        bg = np.ascontiguousarray(bias_eff[n_lo : n_lo + NT])  # (NT, O) fp32
        in_maps.append({"xt": xT, "wk": wk, "cr": cr, "bg": bg})
    return in_maps


def _assemble(results):
    out = np.empty((B, N, O), dtype=np.float32)
    for cid in range(NCORES):
        n_lo = cid * NT
        out[:, n_lo : n_lo + NT, :] = results[cid]["out"].reshape(B, NT, O)
    return out


USE_V2 = False


def _run(x, coef, weight, bias, trace=False, **spmd_kwargs):
    global _BUILT
    from concourse.bass_utils import run_bass_kernel_spmd

    if _BUILT is None:
        _BUILT = _build_bass_v2() if USE_V2 else _build_bass()
    nc = _BUILT
    prep = _prep_inputs_v2 if USE_V2 else _prep_inputs
    asm = _assemble_v2 if USE_V2 else _assemble
    in_maps = prep(x, coef, weight, bias)
    res = run_bass_kernel_spmd(
        nc, in_maps, core_ids=list(range(NCORES)), trace=trace, **spmd_kwargs
    )
    return asm(res.results), res


def kernel(x, coef, weight, bias):
    out, _ = _run(
        np.asarray(x, dtype=np.float32),
        np.asarray(coef, dtype=np.float32),
        np.asarray(weight, dtype=np.float32),
        np.asarray(bias, dtype=np.float32),
    )
    return out


# revision 34
# speedup vs baseline: 120.3453x; 1.0395x over previous
"""Trainium2 Bass kernel for nn_MixtureLinear.

Math: out[b,n,o] = sum_{c,r} x[b,n,c] * coef[n,r] * weight[o,c,r]
                   + sum_r coef[n,r] * bias[o,r]

Strategy (8 NeuronCores, token-parallel):
  - Shard tokens N=1024 into 8 slices of NT=128 tokens; each core computes
    out[:, n_lo:n_hi, :] for all batches B=8 -> 1024 output rows per core.
  - Single fat contraction per core: out[row, o] = sum_K z[K, row] * wk[K, o]
    with K = (r, c) of size R*C = 12288, where
      z[(r,c), row=(b,nl)] = x[b, n_lo+nl, c] * coef[n_lo+nl, r]
      wk[(r,c), o]         = weight[o, c, r]
  - z is built on-chip by the vector engine (fp16, 2x mode) as per-r scaled
    copies of the resident x^T slice; the PE accumulates 96 K-chunks of 128
    into fp32 PSUM. bias term (coef @ bias.T) precomputed on host, added by
    DVE when draining PSUM -> SBUF.

kernel(**inputs) takes the FULL numpy inputs and returns the FULL output.
"""

import sys

import numpy as np

# concourse (Bass/Tile) ships with the container; make sure it resolves even
# from a bare working directory.
for _p in ("/opt/trn_rl_repo", "/root/.axon_site/_ro/trn_rl_repo"):
    try:
        import concourse  # noqa: F401

        break
    except ImportError:
        if _p not in sys.path:
            sys.path.append(_p)

B, N, C, O, R = 8, 1024, 768, 768, 16
NCORES = 8
NT = N // NCORES          # tokens per core
ROWS = B * NT             # output rows per core (b-major: row = b*NT + nl)
KDIM = R * C              # contraction size
NKC = KDIM // 128         # 96 K-chunks of 128
KC_PER_R = C // 128       # 6 chunks per r
GROUPS = 2                # bn-tiles processed in 2 groups of 4 (PSUM capacity)
GW = ROWS // GROUPS       # 512 rows per group

_BUILT = None             # cached (nc,) so repeated kernel() calls reuse program


def _build_bass(reps=None, _timing_shared_lhst=False, _timing_no_zbuild=False):
    import contextlib

    import concourse.mybir as mybir
    from concourse import bacc
    from concourse.tile import TileContext

    fp16 = mybir.dt.float16
    fp32 = mybir.dt.float32

    nc = bacc.Bacc("TRN2", target_bir_lowering=False)

    xT_d = nc.dram_tensor("xt", [C, ROWS], fp16, kind="ExternalInput")
    wk_d = nc.dram_tensor("wk", [KDIM, O], fp16, kind="ExternalInput")
    cr_d = nc.dram_tensor("cr", [128, R * GW], fp16, kind="ExternalInput")
    bg_d = nc.dram_tensor("bg", [NT, O], mybir.dt.float32, kind="ExternalInput")
    out_d = nc.dram_tensor("out", [ROWS, O], fp32, kind="ExternalOutput")

    with TileContext(nc) as tc:
        with (
            tc.tile_pool(name="resident", bufs=1) as rpool,
            tc.tile_pool(name="z", bufs=4) as zpool,
            tc.tile_pool(name="osb", bufs=1) as opool,
            tc.tile_pool(name="psum", bufs=1, space="PSUM") as ppool,
            tc.For_i(0, reps, 1) if reps else contextlib.nullcontext(),
        ):
            # DMA issue order = first-use order (HWDGE ring is FIFO): the PE's
            # kc-th matmul group needs cr[r], xT[cc] (group-0 half) and
            # wk[r][cc]; keep each piece small and just-in-time.
            cr_s = rpool.tile([128, R, GW], fp16, tag="cr")
            crf = cr_d.ap().rearrange("p (r g) -> p r g", g=GW)
            xT_s = rpool.tile([128, C // 128, ROWS], fp16, tag="xT")
            xTr = xT_d.ap().rearrange("(t p) n -> p t n", p=128)
            wkr = wk_d.ap().rearrange("(t p) o -> p t o", p=128)  # [128, 96, O]
            wk_tiles = [
                rpool.tile([128, KC_PER_R, O], fp16, tag=f"wk{i}", name=f"wk_{i}")
                for i in range(R)
            ]

            nc.sync.dma_start(cr_s[:, 0:1, :], crf[:, 0:1, :])
            # group-0 halves of x^T interleaved with the r=0 weight chunks
            for ci in range(C // 128):
                nc.sync.dma_start(
                    xT_s[:, ci : ci + 1, 0:GW], xTr[:, ci : ci + 1, 0:GW]
                )
                nc.sync.dma_start(
                    wk_tiles[0][:, ci : ci + 1, :], wkr[:, ci : ci + 1, :]
                )
            # per-r: coef slice + weight tile, in consumption order. Keep the
            # instruction handles: wk[r>=3] is paced against PE progress below
            # to avoid an HBM burst (2 cores share one HBM stack).
            wk_dmas = {}
            for i in range(1, R):
                nc.sync.dma_start(cr_s[:, i : i + 1, :], crf[:, i : i + 1, :])
                wk_dmas[i] = nc.sync.dma_start(
                    wk_tiles[i], wkr[:, i * KC_PER_R : (i + 1) * KC_PER_R, :]
                )
            # group-1 halves of x^T (needed only after ~kc=96)
            for ci in range(C // 128):
                nc.sync.dma_start(
                    xT_s[:, ci : ci + 1, GW:ROWS], xTr[:, ci : ci + 1, GW:ROWS]
                )
            # bias_eff rows = n_local -> partition dim (needed only at drain)
            bg_s = rpool.tile([NT, O], mybir.dt.float32, tag="bg")
            nc.sync.dma_start(bg_s, bg_d.ap())

            first_mm_of_r = {}
            for g in range(GROUPS):
                psums = [
                    ppool.tile([128, O], fp32, tag=f"ps{t}", name=f"ps_{g}_{t}")
                    for t in range(4)
                ]
                for kc in range(NKC):
                    r, cc = kc // KC_PER_R, kc % KC_PER_R
                    if _timing_no_zbuild:
                        zt = xT_s[:, cc, 0:GW]  # wrong data, timing only
                    else:
                        zt = zpool.tile([128, GW], fp16, tag="z")
                        nc.vector.tensor_mul(
                            zt,
                            xT_s[:, cc, g * GW : (g + 1) * GW],
                            cr_s[:, r, :],
                        )
                    wt = wk_tiles[r][:, cc, :]
                    first, last = kc == 0, kc == NKC - 1
                    for t4 in range(4):
                        lt4 = 0 if _timing_shared_lhst else t4
                        lhsT = zt[:, lt4 * 128 : (lt4 + 1) * 128]
                        mm = nc.tensor.matmul(
                            psums[t4][:, 0:512], lhsT, wt[:, 0:512],
                            start=first, stop=last,
                        )
                        if g == 0 and t4 == 0 and cc == 0:
                            first_mm_of_r[r] = mm
                        nc.tensor.matmul(
                            psums[t4][:, 512:O], lhsT, wt[:, 512:O],
                            start=first, stop=last,
                        )
                for t4 in range(4):
                    # dedicated buffer per output tile: avoids a WAR wait on
                    # the outbound DMA (DVE instructions only have 2 wait slots)
                    osb = opool.tile(
                        [128, O], fp32, tag=f"osb{g}{t4}", name=f"osb_{g}_{t4}"
                    )
                    nc.vector.tensor_add(osb, psums[t4], bg_s)
                    row0 = (g * 4 + t4) * 128
                    nc.sync.dma_start(out_d[row0 : row0 + 128, :], osb)

            # Pace the weight stream: wk[r] may only start once the PE has
            # begun consuming r-3 (stays ~3.6 MB ahead instead of bursting
            # all 18.9 MB against the paired core on the shared HBM stack).
            from concourse.tile import add_dep_helper

            LOOKAHEAD = 3
            for i in range(1 + LOOKAHEAD, R):
                add_dep_helper(
                    wk_dmas[i].ins,
                    first_mm_of_r[i - LOOKAHEAD].ins,
                    sync=True,
                    reason="pace wk stream vs PE progress",
                )

    nc.compile()
    return nc


def _build_bass_v2(reps=None):
    """LDW-amortized variant: stationary = weight chunk (576 LDWEIGHTS,
    1024 moving columns each), output transposed [O, ROWS] (host undoes).
    K is split in 2 halves (h) x o in 2 halves (q); each (h,q) pass keeps
    6 one-bank PSUM tiles [o-128, row-512]; h=0 drains to SBUF partials
    (+bias), h=1 adds partials and stores.
    """
    import contextlib

    import concourse.mybir as mybir
    from concourse import bacc
    from concourse.tile import TileContext

    fp16 = mybir.dt.float16
    fp32 = mybir.dt.float32

    nc = bacc.Bacc("TRN2", target_bir_lowering=False)

    xT_d = nc.dram_tensor("xt", [C, ROWS], fp16, kind="ExternalInput")
    wk_d = nc.dram_tensor("wk", [KDIM, O], fp16, kind="ExternalInput")
    cr_d = nc.dram_tensor("cr", [128, R * ROWS], fp16, kind="ExternalInput")
    bt_d = nc.dram_tensor("bt", [O, ROWS], fp16, kind="ExternalInput")
    out_d = nc.dram_tensor("out", [O, ROWS], fp32, kind="ExternalOutput")

    NOT = O // 128          # 6 o-tiles
    HK = NKC // 2           # 48 kc per K-half
    with TileContext(nc) as tc:
        with (
            tc.tile_pool(name="resident", bufs=1) as rpool,
            tc.tile_pool(name="z", bufs=6) as zpool,
            tc.tile_pool(name="wq", bufs=6) as wpool,
            tc.tile_pool(name="pq", bufs=1) as qpool,
            tc.tile_pool(name="osb", bufs=1) as opool,
            tc.tile_pool(name="psum", bufs=1, space="PSUM") as ppool,
            tc.For_i(0, reps, 1) if reps else contextlib.nullcontext(),
        ):
            crf = cr_d.ap().rearrange("p (r n) -> p r n", n=ROWS)
            cr_s = rpool.tile([128, R, ROWS], fp16, tag="cr")
            nc.sync.dma_start(cr_s[:, 0:1, :], crf[:, 0:1, :])
            xT_s = rpool.tile([128, C // 128, ROWS], fp16, tag="xT")
            xTr = xT_d.ap().rearrange("(t p) n -> p t n", p=128)
            for ci in range(C // 128):
                nc.sync.dma_start(xT_s[:, ci : ci + 1, :], xTr[:, ci : ci + 1, :])
            for i in range(1, R):
                nc.sync.dma_start(cr_s[:, i : i + 1, :], crf[:, i : i + 1, :])
            bt_s = rpool.tile([128, NOT, ROWS], fp16, tag="bt")
            nc.sync.dma_start(bt_s, bt_d.ap().rearrange("(t p) n -> p t n", p=128))

            wkr = wk_d.ap().rearrange("(t p) o -> p t o", p=128)  # [128, 96, O]
            partials = {}
            for h in range(2):
                for q in range(2):
                    ps = {
                        (ot, rh): ppool.tile(
                            [128, 512], fp32, tag=f"ps{ot}{rh}",
                            name=f"ps_{h}_{q}_{ot}_{rh}",
                        )
                        for ot in range(3)
                        for rh in range(2)
                    }
                    for j in range(HK):
                        kc = h * HK + j
                        r, cc = kc // KC_PER_R, kc % KC_PER_R
                        zt = zpool.tile([128, ROWS], fp16, tag="z")
                        nc.vector.tensor_mul(zt, xT_s[:, cc, :], cr_s[:, r, :])
                        wq = wpool.tile([128, 1, 384], fp16, tag="wq")
                        nc.sync.dma_start(
                            wq, wkr[:, kc : kc + 1, q * 384 : (q + 1) * 384]
                        )
                        first, last = j == 0, j == HK - 1
                        for ot in range(3):
                            lhsT = wq[:, 0, ot * 128 : (ot + 1) * 128]
                            for rh in range(2):
                                nc.tensor.matmul(
                                    ps[(ot, rh)], lhsT,
                                    zt[:, rh * 512 : (rh + 1) * 512],
                                    start=first, stop=last,
                                )
                    for ot in range(3):
                        for rh in range(2):
                            bslice = bt_s[
                                :, q * 3 + ot, rh * 512 : (rh + 1) * 512
                            ]
                            if h == 0:
                                pq = qpool.tile(
                                    [128, 512], fp32, tag=f"pq{q}{ot}{rh}",
                                    name=f"pq_{q}_{ot}_{rh}",
                                )
                                nc.vector.tensor_add(pq, ps[(ot, rh)], bslice)
                                partials[(q, ot, rh)] = pq
                            else:
                                osb = opool.tile(
                                    [128, 512], fp32, tag=f"osb{q}{ot}{rh}",
                                    name=f"osb_{q}_{ot}_{rh}",
                                )
                                nc.vector.tensor_add(
                                    osb, ps[(ot, rh)], partials[(q, ot, rh)]
                                )
                                o0 = q * 384 + ot * 128
                                nc.sync.dma_start(
                                    out_d[o0 : o0 + 128,
                                          rh * 512 : (rh + 1) * 512],
                                    osb,
                                )

    nc.compile()
    return nc


NT3 = N // 4            # 256 tokens per core (token quarter)
ROWS3 = B * NT3         # 2048 rows
O3 = O // 2             # 384 out features per core (o half)
NTILE3 = ROWS3 // 128   # 16 row tiles
GROUPS3 = 2             # 8 tiles x 1 PSUM bank per group
GTILES3 = NTILE3 // GROUPS3
GW3 = 128 * GTILES3     # 1024


def _build_bass_v3(reps=None):
    """tokens x4 / O x2 sharding: halves the replicated-weight HBM traffic
    (9.4 MB/core vs 18.9) to cut HBM-stack contention between core pairs.
    Same PE cycle count; 8 one-bank PSUM tiles [128, 384] per group.
    """
    import contextlib

    import concourse.mybir as mybir
    from concourse import bacc
    from concourse.tile import TileContext, add_dep_helper

    fp16 = mybir.dt.float16
    fp32 = mybir.dt.float32

    nc = bacc.Bacc("TRN2", target_bir_lowering=False)

    xT_d = nc.dram_tensor("xt", [C, ROWS3], fp16, kind="ExternalInput")
    wk_d = nc.dram_tensor("wk", [KDIM, O3], fp16, kind="ExternalInput")
    cr_d = nc.dram_tensor("cr", [128, R * GW3], fp16, kind="ExternalInput")
    bg_d = nc.dram_tensor("bg", [NT3, O3], mybir.dt.float32, kind="ExternalInput")
    out_d = nc.dram_tensor("out", [ROWS3, O3], fp32, kind="ExternalOutput")

    with TileContext(nc) as tc:
        with (
            tc.tile_pool(name="resident", bufs=1) as rpool,
            tc.tile_pool(name="z", bufs=4) as zpool,
            tc.tile_pool(name="osb", bufs=1) as opool,
            tc.tile_pool(name="psum", bufs=1, space="PSUM") as ppool,
            tc.For_i(0, reps, 1) if reps else contextlib.nullcontext(),
        ):
            cr_s = rpool.tile([128, R, GW3], fp16, tag="cr")
            crf = cr_d.ap().rearrange("p (r g) -> p r g", g=GW3)
            xT_s = rpool.tile([128, C // 128, ROWS3], fp16, tag="xT")
            xTr = xT_d.ap().rearrange("(t p) n -> p t n", p=128)
            wkr = wk_d.ap().rearrange("(t p) o -> p t o", p=128)  # [128,96,O3]
            wk_tiles = [
                rpool.tile([128, KC_PER_R, O3], fp16, tag=f"wk{i}", name=f"wk_{i}")
                for i in range(R)
            ]

            nc.sync.dma_start(cr_s[:, 0:1, :], crf[:, 0:1, :])
            for ci in range(C // 128):
                nc.sync.dma_start(
                    xT_s[:, ci : ci + 1, 0:GW3], xTr[:, ci : ci + 1, 0:GW3]
                )
                nc.sync.dma_start(
                    wk_tiles[0][:, ci : ci + 1, :], wkr[:, ci : ci + 1, :]
                )
            wk_dmas = {}
            for i in range(1, R):
                nc.sync.dma_start(cr_s[:, i : i + 1, :], crf[:, i : i + 1, :])
                wk_dmas[i] = nc.sync.dma_start(
                    wk_tiles[i], wkr[:, i * KC_PER_R : (i + 1) * KC_PER_R, :]
                )
            for ci in range(C // 128):
                nc.sync.dma_start(
                    xT_s[:, ci : ci + 1, GW3:ROWS3], xTr[:, ci : ci + 1, GW3:ROWS3]
                )
            bg_s = rpool.tile([128, 2, O3], mybir.dt.float32, tag="bg")
            nc.sync.dma_start(bg_s, bg_d.ap().rearrange("(h p) o -> p h o", p=128))

            first_mm_of_r = {}
            for g in range(GROUPS3):
                psums = [
                    ppool.tile([128, O3], fp32, tag=f"ps{t}", name=f"ps_{g}_{t}")
                    for t in range(GTILES3)
                ]
                for kc in range(NKC):
                    r, cc = kc // KC_PER_R, kc % KC_PER_R
                    zt = zpool.tile([128, GW3], fp16, tag="z")
                    nc.vector.tensor_mul(
                        zt, xT_s[:, cc, g * GW3 : (g + 1) * GW3], cr_s[:, r, :]
                    )
                    wt = wk_tiles[r][:, cc, :]
                    first, last = kc == 0, kc == NKC - 1
                    for t8 in range(GTILES3):
                        mm = nc.tensor.matmul(
                            psums[t8], zt[:, t8 * 128 : (t8 + 1) * 128], wt,
                            start=first, stop=last,
                        )
                        if g == 0 and t8 == 0 and cc == 0:
                            first_mm_of_r[r] = mm
                for t8 in range(GTILES3):
                    osb = opool.tile(
                        [128, O3], fp32, tag=f"osb{g}{t8}", name=f"osb_{g}_{t8}"
                    )
                    # tile t8 = (b = t8//2, nl half = t8%2)
                    nc.vector.tensor_add(
                        osb, psums[t8], bg_s[:, t8 % 2, :]
                    )
                    row0 = (g * GTILES3 + t8) * 128
                    nc.sync.dma_start(out_d[row0 : row0 + 128, :], osb)

            LOOKAHEAD = 3
            for i in range(1 + LOOKAHEAD, R):
                add_dep_helper(
                    wk_dmas[i].ins,
                    first_mm_of_r[i - LOOKAHEAD].ins,
                    sync=True,
                    reason="pace wk stream vs PE progress",
                )

    nc.compile()
    return nc


def _prep_inputs_v3(x, coef, weight, bias):
    wkf = np.ascontiguousarray(
        weight.transpose(2, 1, 0).reshape(KDIM, O)
    ).astype(np.float16)
    wk_halves = [
        np.ascontiguousarray(wkf[:, 0:O3]),
        np.ascontiguousarray(wkf[:, O3:O]),
    ]
    bias_eff = (coef @ bias.T).astype(np.float32)  # [N, O]

    in_maps = []
    for cid in range(NCORES):
        tq, oq = cid // 2, cid % 2
        n_lo = tq * NT3
        xs = x[:, n_lo : n_lo + NT3, :]  # (B, NT3, C)
        xT = np.ascontiguousarray(
            xs.transpose(2, 0, 1).reshape(C, ROWS3)
        ).astype(np.float16)
        cf = coef[n_lo : n_lo + NT3].astype(np.float16)  # (NT3, R)
        inner = np.tile(cf.T, (1, GW3 // NT3))  # [R, GW3] (4 b's per group)
        cr = np.ascontiguousarray(
            np.broadcast_to(inner[None, :, :], (128, R, GW3))
        ).reshape(128, R * GW3)
        bg = np.ascontiguousarray(
            bias_eff[n_lo : n_lo + NT3, oq * O3 : (oq + 1) * O3]
        )
        in_maps.append({"xt": xT, "wk": wk_halves[oq], "cr": cr, "bg": bg})
    return in_maps


def _assemble_v3(results):
    out = np.empty((B, N, O), dtype=np.float32)
    for cid in range(NCORES):
        tq, oq = cid // 2, cid % 2
        n_lo = tq * NT3
        out[:, n_lo : n_lo + NT3, oq * O3 : (oq + 1) * O3] = (
            results[cid]["out"].reshape(B, NT3, O3)
        )
    return out


def _prep_inputs_v2(x, coef, weight, bias):
    wk = np.ascontiguousarray(
        weight.transpose(2, 1, 0).reshape(KDIM, O)
    ).astype(np.float16)
    bias_eff = (coef @ bias.T).astype(np.float32)  # [N, O]

    in_maps = []
    for cid in range(NCORES):
        n_lo = cid * NT
        xs = x[:, n_lo : n_lo + NT, :]
        xT = np.ascontiguousarray(
            xs.transpose(2, 0, 1).reshape(C, ROWS)
        ).astype(np.float16)
        cf = coef[n_lo : n_lo + NT].astype(np.float16)  # (NT, R)
        inner = np.tile(cf.T, (1, ROWS // NT))  # [R, ROWS]
        cr = np.ascontiguousarray(
            np.broadcast_to(inner[None, :, :], (128, R, ROWS))
        ).reshape(128, R * ROWS)
        # bias transposed [O, ROWS], rows b-major repeat
        bt = np.ascontiguousarray(
            np.tile(bias_eff[n_lo : n_lo + NT].T, (1, B))
        ).astype(np.float16)
        # note: rows are (b, nl) b-major -> bias pattern repeats per 128: tile
        # along axis1 B times gives [O, B*NT] with [:, b*NT+nl] = bias[nl, :].T
        in_maps.append({"xt": xT, "wk": wk, "cr": cr, "bt": bt})
    return in_maps


def _assemble_v2(results):
    out = np.empty((B, N, O), dtype=np.float32)
    for cid in range(NCORES):
        n_lo = cid * NT
        out[:, n_lo : n_lo + NT, :] = (
            results[cid]["out"].T.reshape(B, NT, O)
        )
    return out


def _prep_inputs(x, coef, weight, bias):
    """Host-side shard + repack. Returns per-core input maps."""
    wk = np.ascontiguousarray(
        weight.transpose(2, 1, 0).reshape(KDIM, O)
    ).astype(np.float16)
    bias_eff = (coef @ bias.T).astype(np.float32)  # [N, O]

    in_maps = []
    for cid in range(NCORES):
        n_lo = cid * NT
        xs = x[:, n_lo : n_lo + NT, :]  # (B, NT, C)
        xT = np.ascontiguousarray(
            xs.transpose(2, 0, 1).reshape(C, ROWS)
        ).astype(np.float16)
        cf = coef[n_lo : n_lo + NT].astype(np.float16)  # (NT, R)
        inner = np.tile(cf.T, (1, GW // NT))  # [R, GW]
        cr = np.ascontiguousarray(
            np.broadcast_to(inner[None, :, :], (128, R, GW))
        )
        bg = np.ascontiguousarray(bias_eff[n_lo : n_lo + NT])  # (NT, O) fp32
        in_maps.append({"xt": xT, "wk": wk, "cr": cr, "bg": bg})
    return in_maps


def _assemble(results):
    out = np.empty((B, N, O), dtype=np.float32)
    for cid in range(NCORES):
        n_lo = cid * NT
        out[:, n_lo : n_lo + NT, :] = results[cid]["out"].reshape(B, NT, O)
    return out


USE_V2 = False


def _run(x, coef, weight, bias, trace=False, **spmd_kwargs):
    global _BUILT
    from concourse.bass_utils import run_bass_kernel_spmd

    if _BUILT is None:
        _BUILT = _build_bass_v2() if USE_V2 else _build_bass()
    nc = _BUILT
    prep = _prep_inputs_v2 if USE_V2 else _prep_inputs
    asm = _assemble_v2 if USE_V2 else _assemble
    in_maps = prep(x, coef, weight, bias)
    res = run_bass_kernel_spmd(
        nc, in_maps, core_ids=list(range(NCORES)), trace=trace, **spmd_kwargs
    )
    return asm(res.results), res


def kernel(x, coef, weight, bias):
    out, _ = _run(
        np.asarray(x, dtype=np.float32),
        np.asarray(coef, dtype=np.float32),
        np.asarray(weight, dtype=np.float32),
        np.asarray(bias, dtype=np.float32),
    )
    return out
